# revision 15
# baseline (speedup 1.0000x reference)
"""Multi-head self-attention (RoPE, causal) Trainium2 kernel — bf16 dataflow.

Tensor-parallel over heads: 16 heads / 8 cores = 2 heads per core
(Megatron-style: Wq/Wk/Wv sharded on output dim, Wo on input dim).
Each core computes a full [S, D] partial of the output projection;
the host sums the 8 partials.

v2 vs baseline: all matmul operands bf16 (fp32_mode=HIGH eliminated,
DMA halved), 1024-wide attention tiles (half the matmul/exp instruction
overheads), masks/copies moved off the scalar engine, output DMA'd
straight from PSUM, software-pipelined emission.

Self-contained: hardcodes all shapes; no sibling imports.
"""

import numpy as np
import ml_dtypes

BF16 = ml_dtypes.bfloat16

S = 4096
D = 1024
DK = 64
NCORES = 8
THETA = 10000.0
CHP = 512           # projection chunk (512-col PSUM granularity)
NCHP = S // CHP     # 8
CHA = 1024          # attention chunk (q columns per chunk)
NCHA = S // CHA     # 4
VS = 132            # v_sb column stride per k-tile: [vA(64) 1A vB(64) 1B pad2]

_CACHE = {}


# ---------------------------------------------------------------------------
# host-side layout helpers
# ---------------------------------------------------------------------------

def _rope_perm64():
    """Permutation of a head's 64 dims so RoPE pairs line up for a
    32-lane stream_shuffle: quadrant q (32 partitions) holds pairs
    16q..16q+15 as [evens(16) | odds(16)]."""
    perm = np.zeros(64, np.int64)
    for d in range(64):
        j, odd = d // 2, d % 2
        pos = 32 * (j // 16) + 16 * odd + (j % 16)
        perm[pos] = d
    return perm


def _trig_tables():
    # partition p: pair index = 16*((p//32)%2) + p%16 ; odd slot if p%32 >= 16
    p = np.arange(128)
    pair = 16 * ((p // 32) % 2) + (p % 16)
    odd = (p % 32) >= 16
    inv_freq = THETA ** (-2.0 * pair / DK)           # [128]
    pos = np.arange(S, dtype=np.float64)
    ang = pos[None, :] * inv_freq[:, None]           # [128, S]
    cos = np.cos(ang).astype(BF16)
    sin = (np.where(odd[:, None], 1.0, -1.0) * np.sin(ang)).astype(BF16)
    return cos, sin


def _host_prep(x, Wq, Wk, Wv, Wo):
    x = np.asarray(x, dtype=np.float32).reshape(S, D)
    Wq = np.asarray(Wq, dtype=np.float32)
    Wk = np.asarray(Wk, dtype=np.float32)
    Wv = np.asarray(Wv, dtype=np.float32)
    Wo = np.asarray(Wo, dtype=np.float32)

    xT = np.ascontiguousarray(x.T).astype(BF16)      # [D, S]
    cos, sin = _trig_tables()
    tri = (np.arange(128)[None, :] >= np.arange(128)[:, None]).astype(BF16)

    perm = _rope_perm64()
    in_maps = []
    for c in range(NCORES):
        hA, hB = 2 * c, 2 * c + 1
        rows_qk = np.concatenate([64 * hA + perm, 64 * hB + perm])
        rows_v = np.arange(128 * c, 128 * c + 128)
        wq_c = np.ascontiguousarray(Wq[rows_qk, :].T).astype(BF16)   # [D, 128]
        wk_c = np.ascontiguousarray(Wk[rows_qk, :].T).astype(BF16)   # [D, 128]
        wv_c = np.ascontiguousarray(Wv[rows_v, :].T).astype(BF16)    # [D, 128]
        wo_c = np.ascontiguousarray(Wo[:, rows_v].T).astype(BF16)    # [128, D]
        in_maps.append({
            "xT": xT, "wq": wq_c, "wk": wk_c, "wv": wv_c, "wo": wo_c,
            "cos": cos, "sin": sin, "tri": tri,
            "ones": np.ones((128, 64), BF16),
            "iden": np.eye(128, dtype=np.float32),
        })
    return in_maps


# ---------------------------------------------------------------------------
# device program
# ---------------------------------------------------------------------------

def _emit(tc, out, xT, wq, wk, wv, wo, cos, sin, tri, ones, iden):
    import concourse.mybir as mybir

    nc = tc.nc
    f32 = mybir.dt.float32
    f32r = mybir.dt.float32r
    bf16 = mybir.dt.bfloat16
    AF = mybir.ActivationFunctionType
    OP = mybir.AluOpType
    SWAP_MASK = [(i + 16) % 32 for i in range(32)]

    with (
        tc.tile_pool(name="consts", bufs=1) as consts,
        tc.tile_pool(name="persist", bufs=1) as persist,
        tc.tile_pool(name="xtp", bufs=2) as xtp,
        tc.tile_pool(name="trig", bufs=2) as trigp,
        tc.tile_pool(name="rope", bufs=2) as ropep,
        tc.tile_pool(name="expp", bufs=3) as expp,
        tc.tile_pool(name="outTp", bufs=2) as outTp,
        tc.tile_pool(name="small", bufs=2) as smallp,
        tc.tile_pool(name="stage", bufs=3) as stagep,
        tc.tile_pool(name="ps_s", bufs=2, space="PSUM") as ps_s,
        tc.tile_pool(name="ps_pv", bufs=1, space="PSUM") as ps_pv,
        tc.tile_pool(name="ps_m", bufs=2, space="PSUM") as ps_m,
    ):
        # ---- constants: spread across engine DMA queues so the startup
        # issue latency (~2us fixed per dma_start on one queue) overlaps.
        # sync gets wq + xt(0) (the critical path to the first matmul).
        wq_sb = consts.tile([128, 1024], bf16)
        nc.sync.dma_start(
            out=wq_sb.rearrange("p (t m) -> p t m", m=128),
            in_=wq.rearrange("(t p) m -> p t m", p=128),
        )
        wk_sb = consts.tile([128, 1024], bf16)
        nc.scalar.dma_start(
            out=wk_sb.rearrange("p (t m) -> p t m", m=128),
            in_=wk.rearrange("(t p) m -> p t m", p=128),
        )
        wv_sb = consts.tile([128, 1024], bf16)
        nc.scalar.dma_start(
            out=wv_sb.rearrange("p (t m) -> p t m", m=128),
            in_=wv.rearrange("(t p) m -> p t m", p=128),
        )
        tri_sb = consts.tile([128, 128], bf16)
        nc.gpsimd.dma_start(out=tri_sb, in_=tri)

        qT_sb = persist.tile([128, S], bf16)   # RoPE'd q, [dk(2 heads), s]
        kT_sb = persist.tile([128, S], bf16)
        v_sb = persist.tile([128, 32 * VS], bf16)
        vv = v_sb.rearrange("p (t c) -> p t c", c=VS)
        ones32 = ones.rearrange("p (t o) -> p t o", o=2)[:, 0:32, :]
        nc.gpsimd.dma_start(out=vv[:, :, 64:65], in_=ones32[:, :, 0:1])
        nc.gpsimd.dma_start(out=vv[:, :, 129:130], in_=ones32[:, :, 1:2])

        id_sb = consts.tile([128, 128], f32r)
        nc.gpsimd.dma_start(out=id_sb, in_=iden)
        ones_c = consts.tile([128, 64], bf16)
        nc.gpsimd.dma_start(out=ones_c, in_=ones)
        ones_sb = ones_c[0:1, :]
        wo_sb = consts.tile([128, 1024], bf16)
        nc.gpsimd.dma_start(out=wo_sb, in_=wo)

        pstate = {}

        # ---- projection units (512-col chunks) -------------------------
        def u0(c):
            """input DMAs for proj chunk c (issued ~1 chunk ahead)."""
            jsl = slice(c * CHP, (c + 1) * CHP)
            xt = xtp.tile([128, 8 * CHP], bf16, tag="xt")
            nc.sync.dma_start(
                out=xt.rearrange("p (t s) -> p t s", s=CHP),
                in_=xT[:, jsl].rearrange("(t p) s -> p t s", p=128),
            )
            cs = trigp.tile([128, CHP], bf16, tag="cs")
            nc.gpsimd.dma_start(out=cs, in_=cos[:, jsl])
            sn = trigp.tile([128, CHP], bf16, tag="sn")
            nc.gpsimd.dma_start(out=sn, in_=sin[:, jsl])
            pstate[("xt", c)] = xt
            pstate[("trig", c)] = (cs, sn)

        def rope(ps, dst, c):
            """dst = ps * cos + swap(ps) * sin  (chunk c)."""
            cs, sn = pstate[("trig", c)]
            sw = ropep.tile([128, CHP], f32, tag="sw")
            nc.vector.stream_shuffle(sw, ps, SWAP_MASK)
            t1 = ropep.tile([128, CHP], f32, tag="t1")
            nc.vector.tensor_tensor(t1, ps, cs, OP.mult)
            t2 = ropep.tile([128, CHP], f32, tag="t2")
            nc.vector.tensor_tensor(t2, sw, sn, OP.mult)
            nc.vector.tensor_tensor(dst, t1, t2, OP.add)

        def u1(c):
            """q projection + RoPE(q)."""
            xt = pstate[("xt", c)]
            psq = ps_m.tile([128, CHP], f32, tag="m")
            for t in range(8):
                nc.tensor.matmul(psq, wq_sb[:, t * 128:(t + 1) * 128],
                                 xt[:, t * CHP:(t + 1) * CHP],
                                 start=(t == 0), stop=(t == 7))
            rope(psq, qT_sb[:, c * CHP:(c + 1) * CHP], c)

        def u2(c):
            """k projection + RoPE(k); prefetch next chunk's inputs."""
            xt = pstate[("xt", c)]
            psk = ps_m.tile([128, CHP], f32, tag="m")
            for t in range(8):
                nc.tensor.matmul(psk, wk_sb[:, t * 128:(t + 1) * 128],
                                 xt[:, t * CHP:(t + 1) * CHP],
                                 start=(t == 0), stop=(t == 7))
            rope(psk, kT_sb[:, c * CHP:(c + 1) * CHP], c)
            del pstate[("trig", c)]
            if c + 1 < NCHP:
                u0(c + 1)

        def u3(c):
            """vT projection + copy to SBUF."""
            xt = pstate.pop(("xt", c))
            psv = ps_m.tile([128, CHP], f32, tag="m")
            for t in range(8):
                nc.tensor.matmul(psv, wv_sb[:, t * 128:(t + 1) * 128],
                                 xt[:, t * CHP:(t + 1) * CHP],
                                 start=(t == 0), stop=(t == 7))
            vt = ropep.tile([128, CHP], f32r, tag="vt")
            nc.vector.tensor_copy(vt, psv)
            pstate[("vt", c)] = vt

        def u4(c):
            """PE-transpose vT -> v tiles in v_sb (bf16, with ones cols)."""
            vt = pstate.pop(("vt", c))
            pst = ps_m.tile([128, CHP], f32r, tag="m")
            for st in range(4):
                nc.tensor.transpose(pst[:, st * 128:(st + 1) * 128],
                                    vt[:, st * 128:(st + 1) * 128], id_sb)
            for st in range(4):
                cb = (4 * c + st) * VS
                nc.vector.tensor_copy(v_sb[:, cb:cb + 64],
                                      pst[:, st * 128:st * 128 + 64])
                nc.vector.tensor_copy(v_sb[:, cb + 65:cb + 129],
                                      pst[:, st * 128 + 64:st * 128 + 128])

        def proj_units(c):
            return [lambda: u1(c), lambda: u2(c), lambda: u3(c), lambda: u4(c)]

        # ---- attention helpers (1024-col chunks, per head) --------------
        def attn_head(J, h, pump, inject=None):
            ktiles = 8 * J + 8
            q0 = J * CHA
            hs = slice(64 * h, 64 * h + 64)
            pv = ps_pv.tile([128, CHA], f32, tag="pv")
            pstate[("pv", J, h)] = pv
            es = {}

            def emit_s(t):
                off = 128 * (t - 8 * J) if t >= 8 * J else 0
                s = ps_s.tile([128, CHA], f32, tag="s")
                kt = kT_sb[hs, t * 128:(t + 1) * 128]
                for h0 in (0, CHP):
                    lo = max(off, h0)
                    hi = h0 + CHP
                    if lo >= hi:
                        continue
                    nc.tensor.matmul(s[:, lo:hi], kt,
                                     qT_sb[hs, q0 + lo:q0 + hi],
                                     start=True, stop=True)
                e = expp.tile([128, CHA], bf16, tag="e")
                nc.scalar.activation(e[:, off:], s[:, off:], AF.Exp, scale=0.125)
                if t >= 8 * J:
                    nc.gpsimd.tensor_tensor(e[:, off:off + 128],
                                            e[:, off:off + 128], tri_sb, OP.mult)
                es[t] = (e, off)

            def emit_pv(t):
                e, off = es.pop(t)
                vt = v_sb[:, t * VS + 65 * h:t * VS + 65 * h + 65]
                for h0 in (0, CHP):
                    lo = max(off, h0)
                    hi = h0 + CHP
                    if lo >= hi:
                        continue
                    last = (t == 8 * J + 3) if h0 == 0 else (t == ktiles - 1)
                    nc.tensor.matmul(pv[0:65, lo:hi], vt, e[:, lo:hi],
                                     start=(t == 0), stop=last,
                                     skip_group_check=True)

            emit_s(0)
            emit_s(1)
            if inject is not None:
                inject()
            for t in range(ktiles):
                if t + 2 < ktiles:
                    emit_s(t + 2)
                emit_pv(t)
                pump()

        def norm_head(J, h, outT):
            """Emit den copy now (DVE); return closure with the PE/DVE tail."""
            pv = pstate.pop(("pv", J, h))
            den = smallp.tile([1, CHA], bf16, tag="den")
            nc.vector.tensor_copy(den, pv[64:65, :])
            hs = slice(64 * h, 64 * h + 64)

            def pe_part():
                # all recip/mult operands kept partition-aligned (0:64);
                # a partition-shifted reciprocal input yields garbage on HW
                rb = smallp.tile([64, CHA], f32, tag="rb")
                for h0 in (0, CHP):
                    rbp = ps_m.tile([128, CHP], f32, tag="m")
                    nc.tensor.matmul(rbp[0:64, :], ones_sb,
                                     den[0:1, h0:h0 + CHP],
                                     start=True, stop=True)
                    nc.vector.reciprocal_approx_fast(out=rb[:, h0:h0 + CHP],
                                                     in_=rbp[0:64, :])
                    nc.vector.tensor_tensor(outT[hs, h0:h0 + CHP],
                                            pv[0:64, h0:h0 + CHP],
                                            rb[:, h0:h0 + CHP], OP.mult)

            return pe_part

        def outproj_unit(J, st, outT):
            r0 = J * CHA + st * 128
            stg = stagep.tile([128, 1024], bf16, tag="stg")
            for oc in range(2):
                ops = ps_m.tile([128, CHP], f32, tag="m")
                nc.tensor.matmul(ops, outT[:, st * 128:(st + 1) * 128],
                                 wo_sb[:, oc * CHP:(oc + 1) * CHP],
                                 start=True, stop=True)
                nc.vector.tensor_copy(stg[:, oc * CHP:(oc + 1) * CHP], ops)
            nc.sync.dma_start(out=out[r0:r0 + 128, :], in_=stg)

        # ---- prologue: warm the ACT tables + first proj chunk ----------
        dummy = smallp.tile([1, 8], bf16, tag="dummy")
        nc.scalar.activation(dummy, ones_c[0:1, 0:8], AF.Exp)
        # attention chunk J spans q columns [J*1024, (J+1)*1024) = proj
        # chunks 2J and 2J+1, so chunks 0 AND 1 must finish before J=0.
        u0(0)
        u1(0)
        u2(0)   # also fires u0(1)
        u3(0)
        u4(0)
        u1(1)
        u2(1)   # also fires u0(2)
        u3(1)
        u4(1)

        # ---- main loop -------------------------------------------------
        outTs = {}
        pending_norm = None      # normB(J-1) PE tail, injected at J head-A
        for J in range(NCHA):
            fill = []
            if J == 0:
                for c in (2, 3):
                    fill += proj_units(c)
            else:
                cs0, cs1 = 2 * J + 2, 2 * J + 3
                if cs0 < NCHP:
                    fill += proj_units(cs0)[:2]
                prev = outTs[J - 1]
                fill += [lambda st=st, o=prev: outproj_unit(J - 1, st, o)
                         for st in range(8)]
                if cs0 < NCHP:
                    fill += proj_units(cs0)[2:]
                if cs1 < NCHP:
                    fill += proj_units(cs1)

            nslots = 2 * (8 * J + 8)
            state = {"slot": 0, "done": 0}

            def pump(fill=fill, nslots=nslots, state=state, J=J):
                state["slot"] += 1
                if J == 0:
                    want = min(state["slot"], len(fill))
                else:
                    want = (state["slot"] * len(fill)) // nslots
                while state["done"] < want:
                    fill[state["done"]]()
                    state["done"] += 1

            outT = outTp.tile([128, CHA], bf16, tag="outT")
            outTs[J] = outT

            attn_head(J, 0, pump, inject=pending_norm)
            normA = norm_head(J, 0, outT)
            attn_head(J, 1, pump, inject=normA)
            pending_norm = norm_head(J, 1, outT)

            # drain any unconsumed fillers (shouldn't happen, but safe)
            while state["done"] < len(fill):
                fill[state["done"]]()
                state["done"] += 1

        # ---- tail ------------------------------------------------------
        pending_norm()
        for st in range(8):
            outproj_unit(NCHA - 1, st, outTs[NCHA - 1])


def _build():
    import concourse.mybir as mybir
    import concourse.tile as tile
    from concourse import bacc

    f32 = mybir.dt.float32
    f32r = mybir.dt.float32r
    bf16 = mybir.dt.bfloat16
    nc = bacc.Bacc("TRN2", target_bir_lowering=False, debug=False,
                   num_devices=NCORES)
    aps = {}
    for name, shape, dt in (
        ("xT", [D, S], bf16), ("wq", [D, 128], bf16), ("wk", [D, 128], bf16),
        ("wv", [D, 128], bf16), ("wo", [128, D], bf16),
        ("cos", [128, S], bf16), ("sin", [128, S], bf16),
        ("tri", [128, 128], bf16), ("ones", [128, 64], bf16),
        ("iden", [128, 128], f32r),
    ):
        aps[name] = nc.dram_tensor(name, shape, dt, kind="ExternalInput").ap()
    out_ap = nc.dram_tensor("out", [S, D], bf16, kind="ExternalOutput").ap()

    with tile.TileContext(nc) as tc:
        _emit(tc, out_ap, aps["xT"], aps["wq"], aps["wk"], aps["wv"],
              aps["wo"], aps["cos"], aps["sin"], aps["tri"], aps["ones"],
              aps["iden"])
    nc.compile()
    return nc


def kernel(x, Wq, Wk, Wv, Wo):
    from concourse.bass_utils import run_bass_kernel_spmd

    if "nc" not in _CACHE:
        _CACHE["nc"] = _build()
    nc = _CACHE["nc"]

    in_maps = _host_prep(x, Wq, Wk, Wv, Wo)
    res = run_bass_kernel_spmd(nc, in_maps, core_ids=list(range(NCORES)))
    acc = np.zeros((S, D), dtype=np.float64)
    for r in res.results:
        acc += np.asarray(r["out"], dtype=np.float64)
    return acc.astype(np.float32).reshape(1, S, D)


# revision 18
# speedup vs baseline: 1.0054x; 1.0054x over previous
"""Multi-head self-attention (RoPE, causal) Trainium2 kernel — bf16 dataflow.

Tensor-parallel over heads: 16 heads / 8 cores = 2 heads per core
(Megatron-style: Wq/Wk/Wv sharded on output dim, Wo on input dim).
Each core computes a full [S, D] partial of the output projection;
the host sums the 8 partials.

v2 vs baseline: all matmul operands bf16 (fp32_mode=HIGH eliminated,
DMA halved), 1024-wide attention tiles (half the matmul/exp instruction
overheads), masks/copies moved off the scalar engine, output DMA'd
straight from PSUM, software-pipelined emission.

Self-contained: hardcodes all shapes; no sibling imports.
"""

import numpy as np
import ml_dtypes

BF16 = ml_dtypes.bfloat16

S = 4096
D = 1024
DK = 64
NCORES = 8
THETA = 10000.0
CHP = 512           # projection chunk (512-col PSUM granularity)
NCHP = S // CHP     # 8
CHA = 1024          # attention chunk (q columns per chunk)
NCHA = S // CHA     # 4
VS = 132            # v_sb column stride per k-tile: [vA(64) 1A vB(64) 1B pad2]

_CACHE = {}


# ---------------------------------------------------------------------------
# host-side layout helpers
# ---------------------------------------------------------------------------

def _rope_perm64():
    """Permutation of a head's 64 dims so RoPE pairs line up for a
    32-lane stream_shuffle: quadrant q (32 partitions) holds pairs
    16q..16q+15 as [evens(16) | odds(16)]."""
    perm = np.zeros(64, np.int64)
    for d in range(64):
        j, odd = d // 2, d % 2
        pos = 32 * (j // 16) + 16 * odd + (j % 16)
        perm[pos] = d
    return perm


def _trig_tables():
    # partition p: pair index = 16*((p//32)%2) + p%16 ; odd slot if p%32 >= 16
    p = np.arange(128)
    pair = 16 * ((p // 32) % 2) + (p % 16)
    odd = (p % 32) >= 16
    inv_freq = THETA ** (-2.0 * pair / DK)           # [128]
    pos = np.arange(S, dtype=np.float64)
    ang = pos[None, :] * inv_freq[:, None]           # [128, S]
    cos = np.cos(ang).astype(BF16)
    sin = (np.where(odd[:, None], 1.0, -1.0) * np.sin(ang)).astype(BF16)
    return cos, sin


def _host_prep(x, Wq, Wk, Wv, Wo):
    x = np.asarray(x, dtype=np.float32).reshape(S, D)
    Wq = np.asarray(Wq, dtype=np.float32)
    Wk = np.asarray(Wk, dtype=np.float32)
    Wv = np.asarray(Wv, dtype=np.float32)
    Wo = np.asarray(Wo, dtype=np.float32)

    xT = np.ascontiguousarray(x.T).astype(BF16)      # [D, S]
    cos, sin = _trig_tables()
    tri = (np.arange(128)[None, :] >= np.arange(128)[:, None]).astype(BF16)

    perm = _rope_perm64()
    in_maps = []
    for c in range(NCORES):
        hA, hB = 2 * c, 2 * c + 1
        rows_qk = np.concatenate([64 * hA + perm, 64 * hB + perm])
        rows_v = np.arange(128 * c, 128 * c + 128)
        wq_c = np.ascontiguousarray(Wq[rows_qk, :].T).astype(BF16)   # [D, 128]
        wk_c = np.ascontiguousarray(Wk[rows_qk, :].T).astype(BF16)   # [D, 128]
        wv_c = np.ascontiguousarray(Wv[rows_v, :].T).astype(BF16)    # [D, 128]
        wo_c = np.ascontiguousarray(Wo[:, rows_v].T).astype(BF16)    # [128, D]
        in_maps.append({
            "xT": xT, "wq": wq_c, "wk": wk_c, "wv": wv_c, "wo": wo_c,
            "cos": cos, "sin": sin, "tri": tri,
            "ones": np.ones((128, 64), BF16),
            "iden": np.eye(128, dtype=np.float32),
        })
    return in_maps


# ---------------------------------------------------------------------------
# device program
# ---------------------------------------------------------------------------

def _emit(tc, out, xT, wq, wk, wv, wo, cos, sin, tri, ones, iden):
    import concourse.mybir as mybir

    nc = tc.nc
    f32 = mybir.dt.float32
    f32r = mybir.dt.float32r
    bf16 = mybir.dt.bfloat16
    AF = mybir.ActivationFunctionType
    OP = mybir.AluOpType
    SWAP_MASK = [(i + 16) % 32 for i in range(32)]

    with (
        tc.tile_pool(name="consts", bufs=1) as consts,
        tc.tile_pool(name="persist", bufs=1) as persist,
        tc.tile_pool(name="xtp", bufs=2) as xtp,
        tc.tile_pool(name="trig", bufs=2) as trigp,
        tc.tile_pool(name="rope", bufs=2) as ropep,
        tc.tile_pool(name="expp", bufs=3) as expp,
        tc.tile_pool(name="outTp", bufs=2) as outTp,
        tc.tile_pool(name="small", bufs=2) as smallp,
        tc.tile_pool(name="stage", bufs=3) as stagep,
        tc.tile_pool(name="ps_s", bufs=2, space="PSUM") as ps_s,
        tc.tile_pool(name="ps_pv", bufs=1, space="PSUM") as ps_pv,
        tc.tile_pool(name="ps_m", bufs=2, space="PSUM") as ps_m,
    ):
        # ---- constants: spread across engine DMA queues so the startup
        # issue latency (~2us fixed per dma_start on one queue) overlaps.
        # sync gets wq + xt(0) (the critical path to the first matmul).
        wq_sb = consts.tile([128, 1024], bf16)
        nc.sync.dma_start(
            out=wq_sb.rearrange("p (t m) -> p t m", m=128),
            in_=wq.rearrange("(t p) m -> p t m", p=128),
        )
        wk_sb = consts.tile([128, 1024], bf16)
        nc.scalar.dma_start(
            out=wk_sb.rearrange("p (t m) -> p t m", m=128),
            in_=wk.rearrange("(t p) m -> p t m", p=128),
        )
        wv_sb = consts.tile([128, 1024], bf16)
        nc.scalar.dma_start(
            out=wv_sb.rearrange("p (t m) -> p t m", m=128),
            in_=wv.rearrange("(t p) m -> p t m", p=128),
        )
        tri_sb = consts.tile([128, 128], bf16)
        nc.gpsimd.dma_start(out=tri_sb, in_=tri)

        qT_sb = persist.tile([128, S], bf16)   # RoPE'd q, [dk(2 heads), s]
        kT_sb = persist.tile([128, S], bf16)
        v_sb = persist.tile([128, 32 * VS], bf16)
        vv = v_sb.rearrange("p (t c) -> p t c", c=VS)
        ones32 = ones.rearrange("p (t o) -> p t o", o=2)[:, 0:32, :]
        nc.gpsimd.dma_start(out=vv[:, :, 64:65], in_=ones32[:, :, 0:1])
        nc.gpsimd.dma_start(out=vv[:, :, 129:130], in_=ones32[:, :, 1:2])

        id_sb = consts.tile([128, 128], f32r)
        nc.gpsimd.dma_start(out=id_sb, in_=iden)
        ones_c = consts.tile([128, 64], bf16)
        nc.gpsimd.dma_start(out=ones_c, in_=ones)
        ones_sb = ones_c[0:1, :]
        wo_sb = consts.tile([128, 1024], bf16)
        nc.gpsimd.dma_start(out=wo_sb, in_=wo)

        pstate = {}

        # ---- projection units (512-col chunks) -------------------------
        def u0(c):
            """input DMAs for proj chunk c (issued ~1 chunk ahead)."""
            jsl = slice(c * CHP, (c + 1) * CHP)
            xt = xtp.tile([128, 8 * CHP], bf16, tag="xt")
            xtr = xt.rearrange("p (t s) -> p t s", s=CHP)
            src = xT[:, jsl].rearrange("(t p) s -> p t s", p=128)
            # split across two DMA queues: halves the serial transfer
            # latency that gates the first projection matmul of the chunk
            nc.sync.dma_start(out=xtr[:, 0:4, :], in_=src[:, 0:4, :])
            nc.scalar.dma_start(out=xtr[:, 4:8, :], in_=src[:, 4:8, :])
            cs = trigp.tile([128, CHP], bf16, tag="cs")
            nc.gpsimd.dma_start(out=cs, in_=cos[:, jsl])
            sn = trigp.tile([128, CHP], bf16, tag="sn")
            nc.gpsimd.dma_start(out=sn, in_=sin[:, jsl])
            pstate[("xt", c)] = xt
            pstate[("trig", c)] = (cs, sn)

        def rope(ps, dst, c):
            """dst = ps * cos + swap(ps) * sin  (chunk c)."""
            cs, sn = pstate[("trig", c)]
            sw = ropep.tile([128, CHP], f32, tag="sw")
            nc.vector.stream_shuffle(sw, ps, SWAP_MASK)
            t1 = ropep.tile([128, CHP], f32, tag="t1")
            nc.vector.tensor_tensor(t1, ps, cs, OP.mult)
            t2 = ropep.tile([128, CHP], f32, tag="t2")
            nc.vector.tensor_tensor(t2, sw, sn, OP.mult)
            nc.vector.tensor_tensor(dst, t1, t2, OP.add)

        def u1(c):
            """q projection + RoPE(q)."""
            xt = pstate[("xt", c)]
            psq = ps_m.tile([128, CHP], f32, tag="m")
            for t in range(8):
                nc.tensor.matmul(psq, wq_sb[:, t * 128:(t + 1) * 128],
                                 xt[:, t * CHP:(t + 1) * CHP],
                                 start=(t == 0), stop=(t == 7))
            rope(psq, qT_sb[:, c * CHP:(c + 1) * CHP], c)

        def u2(c):
            """k projection + RoPE(k); prefetch next chunk's inputs."""
            xt = pstate[("xt", c)]
            psk = ps_m.tile([128, CHP], f32, tag="m")
            for t in range(8):
                nc.tensor.matmul(psk, wk_sb[:, t * 128:(t + 1) * 128],
                                 xt[:, t * CHP:(t + 1) * CHP],
                                 start=(t == 0), stop=(t == 7))
            rope(psk, kT_sb[:, c * CHP:(c + 1) * CHP], c)
            del pstate[("trig", c)]
            if c + 1 < NCHP:
                u0(c + 1)

        def u3(c):
            """vT projection + copy to SBUF."""
            xt = pstate.pop(("xt", c))
            psv = ps_m.tile([128, CHP], f32, tag="m")
            for t in range(8):
                nc.tensor.matmul(psv, wv_sb[:, t * 128:(t + 1) * 128],
                                 xt[:, t * CHP:(t + 1) * CHP],
                                 start=(t == 0), stop=(t == 7))
            vt = ropep.tile([128, CHP], f32r, tag="vt")
            nc.vector.tensor_copy(vt, psv)
            pstate[("vt", c)] = vt

        def u4(c):
            """PE-transpose vT -> v tiles in v_sb (bf16, with ones cols)."""
            vt = pstate.pop(("vt", c))
            pst = ps_m.tile([128, CHP], f32r, tag="m")
            for st in range(4):
                nc.tensor.transpose(pst[:, st * 128:(st + 1) * 128],
                                    vt[:, st * 128:(st + 1) * 128], id_sb)
            for st in range(4):
                cb = (4 * c + st) * VS
                nc.vector.tensor_copy(v_sb[:, cb:cb + 64],
                                      pst[:, st * 128:st * 128 + 64])
                nc.vector.tensor_copy(v_sb[:, cb + 65:cb + 129],
                                      pst[:, st * 128 + 64:st * 128 + 128])

        def proj_units(c):
            return [lambda: u1(c), lambda: u2(c), lambda: u3(c), lambda: u4(c)]

        # ---- attention helpers (1024-col chunks, per head) --------------
        def attn_head(J, h, pump, inject=None):
            ktiles = 8 * J + 8
            q0 = J * CHA
            hs = slice(64 * h, 64 * h + 64)
            pv = ps_pv.tile([128, CHA], f32, tag="pv")
            pstate[("pv", J, h)] = pv
            es = {}

            def emit_s(t):
                off = 128 * (t - 8 * J) if t >= 8 * J else 0
                s = ps_s.tile([128, CHA], f32, tag="s")
                kt = kT_sb[hs, t * 128:(t + 1) * 128]
                for h0 in (0, CHP):
                    lo = max(off, h0)
                    hi = h0 + CHP
                    if lo >= hi:
                        continue
                    nc.tensor.matmul(s[:, lo:hi], kt,
                                     qT_sb[hs, q0 + lo:q0 + hi],
                                     start=True, stop=True)
                e = expp.tile([128, CHA], bf16, tag="e")
                nc.scalar.activation(e[:, off:], s[:, off:], AF.Exp, scale=0.125)
                if t >= 8 * J:
                    nc.gpsimd.tensor_tensor(e[:, off:off + 128],
                                            e[:, off:off + 128], tri_sb, OP.mult)
                es[t] = (e, off)

            def emit_pv(t):
                e, off = es.pop(t)
                vt = v_sb[:, t * VS + 65 * h:t * VS + 65 * h + 65]
                for h0 in (0, CHP):
                    lo = max(off, h0)
                    hi = h0 + CHP
                    if lo >= hi:
                        continue
                    last = (t == 8 * J + 3) if h0 == 0 else (t == ktiles - 1)
                    nc.tensor.matmul(pv[0:65, lo:hi], vt, e[:, lo:hi],
                                     start=(t == 0), stop=last,
                                     skip_group_check=True)

            emit_s(0)
            emit_s(1)
            if inject is not None:
                inject()
            for t in range(ktiles):
                if t + 2 < ktiles:
                    emit_s(t + 2)
                emit_pv(t)
                pump()

        def norm_head(J, h, outT):
            """Emit den copy now (DVE); return closure with the PE/DVE tail."""
            pv = pstate.pop(("pv", J, h))
            den = smallp.tile([1, CHA], bf16, tag="den")
            nc.vector.tensor_copy(den, pv[64:65, :])
            hs = slice(64 * h, 64 * h + 64)

            def pe_part():
                # all recip/mult operands kept partition-aligned (0:64);
                # a partition-shifted reciprocal input yields garbage on HW
                rb = smallp.tile([64, CHA], f32, tag="rb")
                for h0 in (0, CHP):
                    rbp = ps_m.tile([128, CHP], f32, tag="m")
                    nc.tensor.matmul(rbp[0:64, :], ones_sb,
                                     den[0:1, h0:h0 + CHP],
                                     start=True, stop=True)
                    nc.vector.reciprocal_approx_fast(out=rb[:, h0:h0 + CHP],
                                                     in_=rbp[0:64, :])
                    nc.vector.tensor_tensor(outT[hs, h0:h0 + CHP],
                                            pv[0:64, h0:h0 + CHP],
                                            rb[:, h0:h0 + CHP], OP.mult)

            return pe_part

        def outproj_unit(J, st, outT, tail=False):
            r0 = J * CHA + st * 128
            stg = stagep.tile([128, 1024], bf16, tag="stg")
            for oc in range(2):
                ops = ps_m.tile([128, CHP], f32, tag="m")
                nc.tensor.matmul(ops, outT[:, st * 128:(st + 1) * 128],
                                 wo_sb[:, oc * CHP:(oc + 1) * CHP],
                                 start=True, stop=True)
                if tail and oc == 1:
                    # past the last exp, the scalar engine is idle: split
                    # the PSUM drains across ACT+DVE to shorten the tail
                    nc.scalar.activation(stg[:, oc * CHP:(oc + 1) * CHP],
                                         ops, AF.Copy)
                else:
                    nc.vector.tensor_copy(stg[:, oc * CHP:(oc + 1) * CHP],
                                          ops)
            nc.sync.dma_start(out=out[r0:r0 + 128, :], in_=stg)

        # ---- prologue: warm the ACT tables + first proj chunk ----------
        dummy = smallp.tile([1, 8], bf16, tag="dummy")
        nc.scalar.activation(dummy, ones_c[0:1, 0:8], AF.Exp)
        # attention chunk J spans q columns [J*1024, (J+1)*1024) = proj
        # chunks 2J and 2J+1, so chunks 0 AND 1 must finish before J=0.
        u0(0)
        u1(0)
        u2(0)   # also fires u0(1)
        u3(0)
        u4(0)
        u1(1)
        u2(1)   # also fires u0(2)
        u3(1)
        u4(1)

        # ---- main loop -------------------------------------------------
        outTs = {}
        pending_norm = None      # normB(J-1) PE tail, injected at J head-A
        for J in range(NCHA):
            fill = []
            if J == 0:
                for c in (2, 3):
                    fill += proj_units(c)
            else:
                cs0, cs1 = 2 * J + 2, 2 * J + 3
                if cs0 < NCHP:
                    fill += proj_units(cs0)[:2]
                prev = outTs[J - 1]
                fill += [lambda st=st, o=prev: outproj_unit(J - 1, st, o)
                         for st in range(8)]
                if cs0 < NCHP:
                    fill += proj_units(cs0)[2:]
                if cs1 < NCHP:
                    fill += proj_units(cs1)

            nslots = 2 * (8 * J + 8)
            state = {"slot": 0, "done": 0}

            def pump(fill=fill, nslots=nslots, state=state, J=J):
                state["slot"] += 1
                if J == 0:
                    want = min(state["slot"], len(fill))
                else:
                    want = (state["slot"] * len(fill)) // nslots
                while state["done"] < want:
                    fill[state["done"]]()
                    state["done"] += 1

            outT = outTp.tile([128, CHA], bf16, tag="outT")
            outTs[J] = outT

            attn_head(J, 0, pump, inject=pending_norm)
            normA = norm_head(J, 0, outT)
            attn_head(J, 1, pump, inject=normA)
            pending_norm = norm_head(J, 1, outT)

            # drain any unconsumed fillers (shouldn't happen, but safe)
            while state["done"] < len(fill):
                fill[state["done"]]()
                state["done"] += 1

        # ---- tail ------------------------------------------------------
        pending_norm()
        for st in range(8):
            outproj_unit(NCHA - 1, st, outTs[NCHA - 1], tail=True)


def _build():
    import concourse.mybir as mybir
    import concourse.tile as tile
    from concourse import bacc

    f32 = mybir.dt.float32
    f32r = mybir.dt.float32r
    bf16 = mybir.dt.bfloat16
    nc = bacc.Bacc("TRN2", target_bir_lowering=False, debug=False,
                   num_devices=NCORES)
    aps = {}
    for name, shape, dt in (
        ("xT", [D, S], bf16), ("wq", [D, 128], bf16), ("wk", [D, 128], bf16),
        ("wv", [D, 128], bf16), ("wo", [128, D], bf16),
        ("cos", [128, S], bf16), ("sin", [128, S], bf16),
        ("tri", [128, 128], bf16), ("ones", [128, 64], bf16),
        ("iden", [128, 128], f32r),
    ):
        aps[name] = nc.dram_tensor(name, shape, dt, kind="ExternalInput").ap()
    out_ap = nc.dram_tensor("out", [S, D], bf16, kind="ExternalOutput").ap()

    with tile.TileContext(nc) as tc:
        _emit(tc, out_ap, aps["xT"], aps["wq"], aps["wk"], aps["wv"],
              aps["wo"], aps["cos"], aps["sin"], aps["tri"], aps["ones"],
              aps["iden"])
    nc.compile()
    return nc


def kernel(x, Wq, Wk, Wv, Wo):
    from concourse.bass_utils import run_bass_kernel_spmd

    if "nc" not in _CACHE:
        _CACHE["nc"] = _build()
    nc = _CACHE["nc"]

    in_maps = _host_prep(x, Wq, Wk, Wv, Wo)
    res = run_bass_kernel_spmd(nc, in_maps, core_ids=list(range(NCORES)))
    acc = np.zeros((S, D), dtype=np.float64)
    for r in res.results:
        acc += np.asarray(r["out"], dtype=np.float64)
    return acc.astype(np.float32).reshape(1, S, D)
